# revision 4
# baseline (speedup 1.0000x reference)
"""Trainium2 Bass kernel: scatter flat upper-triangular values into dense
[B, 2048, 2048] matrices (zeros below the diagonal).

Strategy (pure data parallel, 4 samples per core on 8 cores):

The output matrix (flattened per sample) is covered by 2048 "band rows":
band row r occupies flat [2049*r, 2049*r + 2048). Band row r holds matrix
row r's triu data (length 2048-r) followed by the zero-prefix of matrix
row r+1 (cols [0, r mod 128) after per-block shortening). Band row starts
are AFFINE (stride 2049), while the input triu row offsets are quadratic
(offset[r] = 2048r - r(r-1)/2) — so input rows are fetched with one
indirect-DMA gather per block (arbitrary per-row element offsets from an
SBUF index table, 128 descriptors per instruction), the ragged junk tail
is zeroed with one affine_select, and the result is stored with one
affine 3-D DMA (row stride 2049). The remaining lower-triangle zeros form
per-block parallelograms with affine row starts (stride 2049) and
constant row length (128k+1), each written by one DMA from an SBUF zero
tile. Every output byte is written exactly once -> no WAW ordering needed.

Per core: 16 blocks x (4 gathers + 1 affine_select + 1 band store
+ 1 zero store) ~= 100 instructions, ~103 MB HBM traffic (~roofline).
"""

import os
import sys

import numpy as np

for _p in ("/opt/trn_rl_repo", "/opt/pypackages"):
    if _p not in sys.path and os.path.isdir(_p):
        sys.path.append(_p)

MAT = 2048
P = 128                      # partitions / rows per block
NB = MAT // P                # 16 blocks
S = 4                        # samples per core
NCORES = 8
BATCH = S * NCORES           # 32
IN_N = MAT * (MAT + 1) // 2  # 2098176 triu elements per sample
PAD = 2048
IN_NP = IN_N + PAD           # padded per-sample input length
OUT_N = MAT * MAT
OUT_NP = OUT_N + PAD         # padded per-sample output length
ZMAX = P * (NB - 1) + 1      # max zero-parallelogram row length (1921)

# knobs for experiments
ZFILL = os.environ.get("TRIU_ZFILL", "1") == "1"   # explicit lower-tri zero DMAs
FOLD = os.environ.get("TRIU_FOLD", "0") == "1"     # one gather per block vs per sample
BUFS = int(os.environ.get("TRIU_BUFS", "3"))

_row_off = None


def _offsets():
    global _row_off
    if _row_off is None:
        r = np.arange(MAT, dtype=np.int64)
        _row_off = r * MAT - r * (r - 1) // 2
    return _row_off


def _build_nc(repeat: int = 1):
    import concourse.bass as bass
    import concourse.tile as tile
    from concourse import bacc, mybir

    nc = bacc.Bacc("TRN2", target_bir_lowering=False, debug=False)
    inp = nc.dram_tensor("inp", [S * IN_NP, 1], mybir.dt.float32, kind="ExternalInput")
    idxt = nc.dram_tensor("idx", [P, NB * S], mybir.dt.int32, kind="ExternalInput")
    out = nc.dram_tensor("out", [S * OUT_NP], mybir.dt.float32, kind="ExternalOutput")

    with tile.TileContext(nc) as tc:
        with (
            tc.tile_pool(name="band", bufs=BUFS) as pool,
            tc.tile_pool(name="const", bufs=1) as cpool,
        ):
            idx_tile = cpool.tile([P, NB * S], mybir.dt.int32)
            nc.sync.dma_start(idx_tile[:], idxt[:, :])
            if ZFILL:
                zt = cpool.tile([P, S * ZMAX], mybir.dt.float32)
                nc.vector.memset(zt[:], 0.0)
            for k in [k for _ in range(repeat) for k in range(NB)]:
                L = MAT - P * k
                t = pool.tile([P, S, L], mybir.dt.float32, tag="band")
                if FOLD:
                    nc.gpsimd.indirect_dma_start(
                        out=t[:],
                        out_offset=None,
                        in_=inp[:],
                        in_offset=bass.IndirectOffsetOnAxis(
                            ap=idx_tile[:, k * S:(k + 1) * S], axis=0
                        ),
                    )
                else:
                    for s in range(S):
                        nc.gpsimd.indirect_dma_start(
                            out=t[:, s],
                            out_offset=None,
                            in_=inp[:],
                            in_offset=bass.IndirectOffsetOnAxis(
                                ap=idx_tile[:, k * S + s:k * S + s + 1], axis=0
                            ),
                        )
                # keep element (p, s, l) iff l < L - p (the row's data length)
                nc.gpsimd.affine_select(
                    out=t[:],
                    in_=t[:],
                    compare_op=mybir.AluOpType.is_gt,
                    fill=0.0,
                    base=L,
                    pattern=[[0, S], [-1, L]],
                    channel_multiplier=-1,
                )
                # band store: band row p -> flat 2049*(128k+p), per sample
                oap = bass.AP(
                    out, (MAT + 1) * P * k, [[MAT + 1, P], [OUT_NP, S], [1, L]]
                )
                nc.sync.dma_start(out=oap, in_=t[:])
                if ZFILL:
                    # zero parallelogram: matrix rows R=128k+1+j (j<cnt),
                    # cols [R-1-128k, R-1], length 128k+1, row starts affine
                    zl = P * k + 1
                    cnt = P if k < NB - 1 else P - 1
                    zap = bass.AP(
                        out,
                        (P * k + 1) * MAT,
                        [[MAT + 1, cnt], [OUT_NP, S], [1, zl]],
                    )
                    nc.scalar.dma_start(out=zap, in_=zt[:cnt, :S * zl])
    nc.compile()
    return nc


_NC = None


def _get_nc():
    global _NC
    if _NC is None:
        _NC = _build_nc()
    return _NC


def make_in_maps(inputs: np.ndarray):
    """Shard + pad the [32, IN_N] input into 8 per-core in_maps."""
    assert inputs.shape == (BATCH, IN_N), inputs.shape
    x = np.ascontiguousarray(inputs, dtype=np.float32)
    xp = np.zeros((BATCH, IN_NP), dtype=np.float32)
    xp[:, :IN_N] = x
    xp = xp.reshape(NCORES, S * IN_NP)

    off = _offsets()
    idx = np.zeros((P, NB * S), dtype=np.int32)
    for k in range(NB):
        for s in range(S):
            idx[:, k * S + s] = (off[k * P:(k + 1) * P] + s * IN_NP).astype(np.int32)
    return [{"inp": xp[c][:, None], "idx": idx} for c in range(NCORES)]


def assemble_out(results) -> np.ndarray:
    outs = []
    for c in range(NCORES):
        o = results[c]["out"].reshape(S, OUT_NP)[:, :OUT_N]
        outs.append(o.reshape(S, MAT, MAT))
    return np.concatenate(outs, axis=0)


def kernel(inputs: np.ndarray) -> np.ndarray:
    from concourse.bass_utils import run_bass_kernel_spmd

    nc = _get_nc()
    in_maps = make_in_maps(np.asarray(inputs))
    res = run_bass_kernel_spmd(nc, in_maps, core_ids=list(range(NCORES)))
    return assemble_out(res.results)


if __name__ == "__main__":
    rng = np.random.default_rng(0)
    x = rng.standard_normal((BATCH, IN_N), dtype=np.float32)
    y = kernel(x)
    # numpy reference
    r, c = np.triu_indices(MAT)
    exp = np.zeros((BATCH, MAT, MAT), dtype=np.float32)
    exp[:, r, c] = x
    err = np.abs(y - exp).max()
    denom = max(np.abs(exp).max(), 1e-9)
    print("max abs err:", err, "rel:", err / denom)
    assert err == 0.0, "mismatch"
    print("OK")
